# revision 13
# baseline (speedup 1.0000x reference)
"""Trainium2 Bass kernel for nn_BasicBlockOurIn (sparse-conv BasicBlock).

Computation (see problem reference):
    out = lrelu(inorm2(conv(lrelu(inorm1(conv(f, w1))), w2)) + f)
where conv is a 27-tap kernel-map sparse convolution, inorm is per-batch-
instance instance norm (unbiased var), lrelu slope 0.01.

Sharding: batch_ids are sorted with exactly 8192 points per instance and the
kernel map never crosses instances, so each of the 8 NeuronCores gets one
batch instance -- fully independent, no collectives.

Per-core algorithm (fp16 data, fp32 PSUM/stats), all feature-major in SBUF:
  - The input rows are DMA-xbar-transposed into ft_f [C, PER].
  - The center (identity) tap is computed W-stationary straight into PSUM
    octant tiles: psum[:, oct] = W_id^T @ ft_f[:, oct].
  - The ~900 valid non-center edges ("tokens") are gathered once from the
    input rows in (dst-octant, tap) order (uniform layout across cores,
    octants padded to 128-token chunks so chunk <-> 512-dst octant align).
    Per-tap-run matmuls produce Yft [C, tok] in PSUM; a PE transpose gives
    token-rows Ysb; one-hot scatter matmuls (rhs = per-core D blocks)
    accumulate each chunk's contributions into its octant's PSUM tile, so
    sparse and dense meet in PSUM with zero DMA traffic.
  - Instance-norm stats come from fused accumulators: the PSUM->SBUF
    evacuation computes sum(y) (activation/tensor-scalar accum_out) and one
    DVE pass computes sum(y^2); mean/var/1/std are [C,1] ops. The scalar
    engine only ever runs Copy/Identity/Sqrt (one act-table load); every
    leaky-ReLU is max(x, 0.01x) on the DVE in its 4x fp16 mode.
  - conv2 needs a1[:, src(t)] for the same token set; instead of a DRAM
    round-trip + gather, it is computed compactly on the PE:
    y1[:, src(t)] = W1id^T @ G1[:, t] + sum_{t': dst(t')=src(t)} Y1[:, t']
    (the second term is a banded compact-to-compact one-hot matmul C),
    then the instance-norm affine + lrelu is applied to the compact tile.
  - Tail fuses residual + affine + lrelu on Act/DVE and streams the output.
"""

import sys

if "/opt/trn_rl_repo" not in sys.path:
    sys.path.insert(0, "/opt/trn_rl_repo")

import numpy as np

N = 65536
C = 128
B = 8
PER = 8192
KVOL = 27
P = 128
NCORES = 8
EPS = 1e-6
NEG_SLOPE = 0.01
OCT = 512                    # dst-octant size; chunk(128 tokens) <-> octant
NOCT = PER // OCT            # 16
ZROW = PER                   # all-zero pad row in the DRAM row table
NROWS = PER + 1

_prog_cache = {}
_last_results = None


# --------------------------------------------------------------------------
# host-side planning
# --------------------------------------------------------------------------

def _build_plan(nbr):
    """Analyze neighbor_idx; return None if sharding assumptions fail.

    Token layout (uniform across cores): for each dst-octant o (PER/OCT of
    them) and tap k, a segment of seglen[o,k] = max over cores of that
    (octant, tap) population; octant totals padded to a multiple of 128 so
    every 128-token chunk belongs to exactly one octant.
    """
    identity_ks = []
    arange_n = np.arange(N, dtype=np.int64)
    for k in range(KVOL):
        if np.array_equal(nbr[k], arange_n):
            identity_ks.append(k)

    percore = []
    for c in range(NCORES):
        sl = nbr[:, c * PER:(c + 1) * PER].astype(np.int64)
        v = sl >= 0
        loc = sl - c * PER
        if ((loc < 0) | (loc >= PER))[v].any():
            return None  # non-local neighbor: fall back
        toks = []
        for k in range(KVOL):
            if k in identity_ks:
                continue
            dsts = np.nonzero(v[k])[0]
            for d in dsts:
                toks.append((k, int(d), int(loc[k, d])))
        percore.append(toks)

    sparse_ks = sorted({k for toks in percore for (k, _, _) in toks})
    nsp = len(sparse_ks)
    wi_of = {k: i for i, k in enumerate(sparse_ks)}
    if not nsp:
        return None  # degenerate; fall back

    # uniform segment lengths
    seglen = {}
    for c in range(NCORES):
        cnt = {}
        for (k, d, s) in percore[c]:
            key = (d // OCT, k)
            cnt[key] = cnt.get(key, 0) + 1
        for key, n in cnt.items():
            seglen[key] = max(seglen.get(key, 0), n)

    off = {}
    runs = []           # (wi, off, len) for Y matmuls (split at 512 banks)
    cur = 0
    oct_chunks = {}     # octant -> (chunk_lo, chunk_hi)
    for o in range(NOCT):
        ostart = cur
        for k in sparse_ks:
            if (o, k) not in seglen:
                continue
            ln = seglen[(o, k)]
            off[(o, k)] = cur
            # split runs at 512-col PSUM bank boundaries
            a = cur
            while a < cur + ln:
                b = min(cur + ln, (a // 512 + 1) * 512)
                runs.append((wi_of[k], a, b - a))
                a = b
            cur += ln
        pad = (-cur) % 128
        if pad:
            runs.append((0, cur, pad))   # gap filler: reads zero G columns
            cur += pad
        if cur == ostart:                # empty octant: keep alignment
            runs.append((0, cur, 128))
            cur += 128
        oct_chunks[o] = (ostart // 128, cur // 128)
    npad = cur
    assert npad % 128 == 0
    nchunks = npad // 128

    # per-core token positions + gather sources
    gsrc = np.full((NCORES, npad), ZROW, dtype=np.int64)
    tokpos = []
    for c in range(NCORES):
        cnt = {}
        pos = []
        for (k, d, s) in percore[c]:
            key = (d // OCT, k)
            p = off[key] + cnt.get(key, 0)
            cnt[key] = cnt.get(key, 0) + 1
            pos.append(p)
            gsrc[c, p] = s
        tokpos.append(pos)

    # D scatter blocks: (chunk, dst window of 128) union over cores
    dset = set()
    for c in range(NCORES):
        for (k, d, s), p in zip(percore[c], tokpos[c]):
            dset.add((p // 128, d // 128))
    dblocks = sorted(dset, key=lambda t: (t[1], t[0]))
    dindex = {blk: i for i, blk in enumerate(dblocks)}
    nD = len(dblocks)
    dmat = np.zeros((NCORES, nD, P, P), dtype=np.float16)
    for c in range(NCORES):
        for (k, d, s), p in zip(percore[c], tokpos[c]):
            bi = dindex[(p // 128, d // 128)]
            dmat[c, bi, p % 128, d % 128] = 1.0
    # scatter schedule per dst window: list of (chunk, block idx)
    win_sc = [[] for _ in range(PER // 128)]
    for (tc, w), bi in dindex.items():
        win_sc[w].append((tc, bi))

    # C correction blocks: C[t', t] = 1 iff dst(t') == src(t)
    cset = set()
    centries = []  # (core, p', p)
    for c in range(NCORES):
        dstpos = {}
        for (k, d, s), p in zip(percore[c], tokpos[c]):
            dstpos.setdefault(d, []).append(p)
        for (k, d, s), p in zip(percore[c], tokpos[c]):
            for pp in dstpos.get(s, []):
                cset.add((pp // 128, p // 128))
                centries.append((c, pp, p))
    cblocks = sorted(cset, key=lambda t: (t[1], t[0]))
    cindex = {blk: i for i, blk in enumerate(cblocks)}
    nC = len(cblocks)
    cmat = np.zeros((NCORES, max(nC, 1), P, P), dtype=np.float16)
    for (c, pp, p) in centries:
        cmat[c, cindex[(pp // 128, p // 128)], pp % 128, p % 128] = 1.0
    # correction schedule per token chunk: list of (src chunk tpc, block idx)
    tc_sc = [[] for _ in range(nchunks)]
    for (tpc, tc), ci in cindex.items():
        tc_sc[tc].append((tpc, ci))

    return dict(identity_ks=identity_ks, sparse_ks=sparse_ks, runs=runs,
                npad=npad, nchunks=nchunks, nD=nD, nC=nC,
                win_sc=win_sc, tc_sc=tc_sc,
                gsrc=gsrc, dmat=dmat, cmat=cmat)


def _wrap16(idx_1d):
    """[M] logical -> [128, M//16] wrapped int16 layout for dma_gather."""
    m = idx_1d.shape[0]
    a = idx_1d.reshape(m // 16, 16).T          # [16, m//16]
    return np.tile(a, (8, 1)).astype(np.int16)


# --------------------------------------------------------------------------
# device program
# --------------------------------------------------------------------------

def _build_nc(runs, npad, nchunks, nD, nC, win_sc, tc_sc, nsp, ablate=()):
    import concourse.bacc as bacc
    import concourse.tile as tile
    from concourse import mybir

    FP16 = mybir.dt.float16
    FP32 = mybir.dt.float32
    I16 = mybir.dt.int16
    Copy = mybir.ActivationFunctionType.Copy
    Ident = mybir.ActivationFunctionType.Identity
    Sqrt = mybir.ActivationFunctionType.Sqrt
    MULT = mybir.AluOpType.mult
    ADD = mybir.AluOpType.add
    MAX = mybir.AluOpType.max

    NW = PER // 128          # dst windows (64)
    TAIL = 2048              # tail/apply chunk width
    EV = 512                 # dense evac width (one psum tile)

    nc = bacc.Bacc(None, target_bir_lowering=False, debug=False,
                   num_swdge_queues=4)
    with tile.TileContext(nc) as tc:
        with tc.tile_pool(name="sing", bufs=1) as sing, \
             tc.tile_pool(name="dps", bufs=2, space="PSUM") as dps, \
             tc.tile_pool(name="ysbps", bufs=1, space="PSUM") as ysbps, \
             tc.tile_pool(name="bigps", bufs=1, space="PSUM") as bigps:

            rows1 = nc.dram_tensor("rows1", [NROWS, P], FP16,
                                   kind="ExternalInput")[:]
            wid = [nc.dram_tensor(f"wid{i}", [P, P], FP16,
                                  kind="ExternalInput")[:] for i in (1, 2)]
            wsp = [nc.dram_tensor(f"wsp{i}", [P, nsp, P], FP16,
                                  kind="ExternalInput")[:] for i in (1, 2)]
            dmat = nc.dram_tensor("dmat", [P, nD, P], FP16,
                                  kind="ExternalInput")[:]
            cmat = nc.dram_tensor("cmat", [P, nC, P], FP16,
                                  kind="ExternalInput")[:]
            gidx = nc.dram_tensor("gidx", [P, npad // 16], I16,
                                  kind="ExternalInput")[:]
            ident = nc.dram_tensor("ident", [P, P], FP16,
                                   kind="ExternalInput")[:]
            gb = [nc.dram_tensor(nm, [P, 1], FP32, kind="ExternalInput")[:]
                  for nm in ("gam1", "bet1", "gam2", "bet2")]
            out_ft = nc.dram_tensor("out_ft", [P, PER], FP16,
                                    kind="ExternalOutput")[:]

            # ---- loads (ordered by need on the single DMA device) ----
            ft_f = sing.tile([P, PER], FP16, tag="ftf")
            QT = PER // 4
            for q in range(2):
                nc.sync.dma_start_transpose(
                    ft_f[:, q * QT:(q + 1) * QT],
                    rows1[q * QT:(q + 1) * QT, :])
            gidx_sb = sing.tile([P, npad // 16], I16, tag="gidx")
            nc.sync.dma_start(gidx_sb[:], gidx)
            g1 = sing.tile([P, npad], FP16, tag="g1")
            if "gather" in ablate:
                nc.vector.memset(g1[:], 0.0)
            for q in (() if "gather" in ablate else (0, 1)):
                h = npad // 2
                nc.gpsimd.dma_gather(
                    out_ap=g1[:, q * h:(q + 1) * h]
                        .rearrange("p (o m) -> p o m", o=1),
                    in_ap=rows1,
                    idxs_ap=gidx_sb[:, q * h // 16:(q + 1) * h // 16],
                    num_idxs=h, num_idxs_reg=h, elem_size=P,
                    transpose=True, queue_num=q)
            wsp_sb, wid_sb = [], []
            s = sing.tile([P, nsp, P], FP16, tag="wsp1")
            nc.sync.dma_start(s[:], wsp[0])
            wsp_sb.append(s)
            s = sing.tile([P, P], FP16, tag="wid1")
            nc.sync.dma_start(s[:], wid[0])
            wid_sb.append(s)
            ident_sb = sing.tile([P, P], FP16, tag="ident")
            nc.sync.dma_start(ident_sb[:], ident)
            for q in range(2, 4):
                nc.sync.dma_start_transpose(
                    ft_f[:, q * QT:(q + 1) * QT],
                    rows1[q * QT:(q + 1) * QT, :])
            d_sb = sing.tile([P, nD, P], FP16, tag="dsb")
            nc.sync.dma_start(d_sb[:], dmat)
            c_sb = sing.tile([P, nC, P], FP16, tag="csb")
            nc.sync.dma_start(c_sb[:], cmat)
            s = sing.tile([P, P], FP16, tag="wid2")
            nc.sync.dma_start(s[:], wid[1])
            wid_sb.append(s)
            s = sing.tile([P, nsp, P], FP16, tag="wsp2")
            nc.sync.dma_start(s[:], wsp[1])
            wsp_sb.append(s)
            gb_sb = []
            for i, t in enumerate(gb):
                s = sing.tile([P, 1], FP32, name=f"gb{i}", tag=f"gb{i}")
                nc.sync.dma_start(s[:], t)
                gb_sb.append(s)
            eps_sb = sing.tile([P, 1], FP32, tag="eps")
            nc.vector.memset(eps_sb[:], EPS)
            zeros = sing.tile([P, EV], FP16, tag="zeros")
            nc.vector.memset(zeros[:], 0.0)

            # working SBUF tiles
            yftsb = [sing.tile([P, npad], FP16, name=f"yftsb{i}",
                               tag=f"yftsb{i}") for i in range(2)]
            ysb = [sing.tile([P, nchunks, P], FP16, name=f"ysb{i}",
                             tag=f"ysb{i}") for i in range(2)]
            g2raw = sing.tile([P, npad], FP16, tag="g2raw")
            g2 = sing.tile([P, npad], FP16, tag="g2")
            y_sb = [sing.tile([P, PER], FP16, name=f"ysum{i}",
                              tag=f"ysum{i}") for i in range(2)]  # y1, y2
            a1 = sing.tile([P, PER], FP16, tag="a1")
            osb = sing.tile([P, PER], FP16, tag="osb")
            zscr = sing.tile([P, TAIL], FP16, tag="zscr")
            qscr = sing.tile([P, TAIL], FP16, tag="qscr")
            sy = [sing.tile([P, PER // EV], FP32, name=f"sy{i}",
                            tag=f"sy{i}") for i in range(2)]
            sq = [sing.tile([P, PER // EV], FP32, name=f"sq{i}",
                            tag=f"sq{i}") for i in range(2)]
            sstat = [sing.tile([P, 1], FP32, name=f"sch{i}", tag=f"sch{i}")
                     for i in range(2)]
            bstat = [sing.tile([P, 1], FP32, name=f"bch{i}", tag=f"bch{i}")
                     for i in range(2)]

            def small(tag):
                return sing.tile([P, 1], FP32, name=tag, tag=tag)

            def sparse_chain(i, G):
                """gather-side compute for conv i: Yft -> Ysb (token rows)."""
                yft_ps = bigps.tile([P, npad], FP32, tag="big")
                if "y" in ablate:
                    nc.vector.memset(yft_ps[:], 0.0)
                for (wi, o, ln) in ([] if "y" in ablate else runs):
                    nc.tensor.matmul(out=yft_ps[:, o:o + ln],
                                     lhsT=wsp_sb[i][:, wi, :],
                                     rhs=G[:, o:o + ln],
                                     start=True, stop=True)
                nc.scalar.activation(yftsb[i][:], yft_ps[:], Copy)
                ysb_ps = ysbps.tile([P, nchunks, P], FP16, tag="ysbp")
                if "transp" in ablate:
                    nc.vector.memset(ysb_ps[:], 0.0)
                for ch in range(0 if "transp" in ablate else nchunks):
                    nc.tensor.transpose(
                        out=ysb_ps[:, ch, :],
                        in_=yftsb[i][:, ch * P:(ch + 1) * P],
                        identity=ident_sb[:])
                nc.scalar.activation(ysb[i][:], ysb_ps[:], Copy)

            def dense_scatter(i, rhs_sb):
                """dense + scatter into octant PSUM tiles; evac + stats."""
                for t in range(PER // EV):          # 8 tiles of [P, 1024]
                    pt = dps.tile([P, EV], FP32, tag="dp")
                    for w in range(t * (EV // 128), (t + 1) * (EV // 128)):
                        scs = [] if "scatter" in ablate else win_sc[w]
                        o0 = (w * 128) % EV
                        c0 = w * 128
                        nc.tensor.matmul(out=pt[:, o0:o0 + 128],
                                         lhsT=wid_sb[i][:],
                                         rhs=rhs_sb[:, c0:c0 + 128],
                                         start=True, stop=(len(scs) == 0))
                        for j, (tcn, bi) in enumerate(scs):
                            nc.tensor.matmul(
                                out=pt[:, o0:o0 + 128],
                                lhsT=ysb[i][:, tcn, :],
                                rhs=d_sb[:, bi, :],
                                start=False, stop=(j == len(scs) - 1))
                    # evacuate + accumulate sum(y); alternate Act / DVE
                    dst = y_sb[i][:, t * EV:(t + 1) * EV]
                    if t % 4 != 3:
                        nc.scalar.activation(dst, pt[:], Copy,
                                             accum_out=sy[i][:, t:t + 1])
                    else:
                        nc.vector.scalar_tensor_tensor(
                            out=dst, in0=pt[:], scalar=1.0,
                            in1=zeros[:], op0=MULT, op1=ADD,
                            accum_out=sy[i][:, t:t + 1])
                # sum(y^2) from the fp16 copy (DVE 4x)
                for t in range(PER // EV):
                    ys = y_sb[i][:, t * EV:(t + 1) * EV]
                    nc.vector.scalar_tensor_tensor(
                        out=qscr[:, :EV], in0=ys, scalar=1.0,
                        in1=ys, op0=MULT, op1=MULT,
                        accum_out=sq[i][:, t:t + 1])

            def stats(i):
                ssum = small(f"ssum{i}")
                nc.vector.reduce_sum(out=ssum[:], in_=sy[i][:],
                                     axis=mybir.AxisListType.X)
                qsum = small(f"qsum{i}")
                nc.vector.reduce_sum(out=qsum[:], in_=sq[i][:],
                                     axis=mybir.AxisListType.X)
                mneg = small(f"mneg{i}")
                nc.vector.tensor_scalar(out=mneg[:], in0=ssum[:],
                                        scalar1=-1.0 / PER, scalar2=None,
                                        op0=MULT)
                qq = small(f"qq{i}")
                nc.vector.scalar_tensor_tensor(
                    out=qq[:], in0=ssum[:], scalar=mneg[:], in1=qsum[:],
                    op0=MULT, op1=ADD)
                std = small(f"std{i}")
                nc.scalar.activation(out=std[:], in_=qq[:], func=Sqrt,
                                     bias=eps_sb[:], scale=1.0 / (PER - 1))
                rstd = small(f"rstd{i}")
                nc.vector.reciprocal(out=rstd[:], in_=std[:])
                nc.vector.tensor_tensor(out=sstat[i][:], in0=gb_sb[2 * i][:],
                                        in1=rstd[:], op=MULT)
                mean = small(f"mean{i}")
                nc.vector.tensor_scalar(out=mean[:], in0=mneg[:],
                                        scalar1=-1.0, scalar2=None, op0=MULT)
                nc.vector.ln_bwd_dx(bstat[i][:], gb_sb[2 * i + 1][:],
                                    mean[:], sstat[i][:], 0.0, 1.0)

            # ================= conv1 =================
            sparse_chain(0, g1)

            # G2 compact pre-norm: W1id^T G1 + C-blocks from Ysb1
            g2ps = bigps.tile([P, npad], FP32, tag="big")
            if "g2" in ablate:
                nc.vector.memset(g2ps[:], 0.0)
            for tcn in range(0 if "g2" in ablate else nchunks):
                o0 = tcn * P
                nc.tensor.matmul(out=g2ps[:, o0:o0 + P],
                                 lhsT=wid_sb[0][:], rhs=g1[:, o0:o0 + P],
                                 start=True, stop=(len(tc_sc[tcn]) == 0))
                for j, (tpc, ci) in enumerate(tc_sc[tcn]):
                    nc.tensor.matmul(out=g2ps[:, o0:o0 + P],
                                     lhsT=ysb[0][:, tpc, :],
                                     rhs=c_sb[:, ci, :],
                                     start=False,
                                     stop=(j == len(tc_sc[tcn]) - 1))
            nc.scalar.activation(g2raw[:], g2ps[:], Copy)

            dense_scatter(0, ft_f)
            stats(0)

            # apply a1 = lrelu(s1*y1 + b1)  (DVE 4x, chunked)
            for t in range(PER // TAIL):
                ys = y_sb[0][:, t * TAIL:(t + 1) * TAIL]
                nc.vector.tensor_scalar(out=zscr[:], in0=ys,
                                        scalar1=sstat[0][:],
                                        scalar2=bstat[0][:],
                                        op0=MULT, op1=ADD)
                nc.vector.scalar_tensor_tensor(
                    out=a1[:, t * TAIL:(t + 1) * TAIL], in0=zscr[:],
                    scalar=NEG_SLOPE, in1=zscr[:], op0=MULT, op1=MAX)

            # G2 = lrelu(s1*g2raw + b1)
            for t in range(npad // TAIL):
                gr = g2raw[:, t * TAIL:(t + 1) * TAIL]
                nc.vector.tensor_scalar(out=zscr[:], in0=gr,
                                        scalar1=sstat[0][:],
                                        scalar2=bstat[0][:],
                                        op0=MULT, op1=ADD)
                nc.vector.scalar_tensor_tensor(
                    out=g2[:, t * TAIL:(t + 1) * TAIL], in0=zscr[:],
                    scalar=NEG_SLOPE, in1=zscr[:], op0=MULT, op1=MAX)

            # ================= conv2 =================
            sparse_chain(1, g2)
            dense_scatter(1, a1)
            stats(1)

            # tail: out = lrelu(s2*y2 + b2 + ft); double-buffered scratch so
            # the Act affine of chunk t+1 overlaps the DVE add/lrelu of t
            for t in range(PER // TAIL):
                sl = slice(t * TAIL, (t + 1) * TAIL)
                buf = zscr if t % 2 == 0 else qscr
                nc.scalar.activation(out=buf[:], in_=y_sb[1][:, sl],
                                     func=Ident, bias=bstat[1][:],
                                     scale=sstat[1][:])
                nc.vector.tensor_tensor(out=buf[:], in0=buf[:],
                                        in1=ft_f[:, sl], op=ADD)
                nc.vector.scalar_tensor_tensor(
                    out=osb[:, sl], in0=buf[:], scalar=NEG_SLOPE,
                    in1=buf[:], op0=MULT, op1=MAX)
                nc.sync.dma_start(out_ft[:, sl], osb[:, sl])

    nc.compile()
    return nc


# --------------------------------------------------------------------------
# numpy fallback (only used if sharding assumptions fail)
# --------------------------------------------------------------------------

def _numpy_ref(feats, batch_ids, neighbor_idx, w1, gamma1, beta1,
               w2, gamma2, beta2):
    f = feats.astype(np.float64)

    def conv(x, w):
        out = np.zeros((x.shape[0], w.shape[-1]), dtype=np.float64)
        for k in range(KVOL):
            idx = neighbor_idx[k]
            g = np.where((idx >= 0)[:, None], x[np.maximum(idx, 0)], 0.0)
            out += g @ w[k]
        return out

    def inorm(x, gamma, beta):
        out = np.empty_like(x)
        for b in range(B):
            m = batch_ids == b
            xb = x[m]
            cnt = xb.shape[0]
            mean = xb.mean(axis=0)
            var = ((xb * xb).sum(0) - cnt * mean * mean) / (cnt - 1.0) + EPS
            out[m] = (xb - mean) / np.sqrt(var)
        return out * gamma + beta

    def leaky(x):
        return np.where(x >= 0, x, NEG_SLOPE * x)

    out = leaky(inorm(conv(f, w1.astype(np.float64)), gamma1, beta1))
    out = inorm(conv(out, w2.astype(np.float64)), gamma2, beta2)
    out = leaky(out + f)
    return out.astype(np.float32)


# --------------------------------------------------------------------------
# entry point
# --------------------------------------------------------------------------

def kernel(feats, batch_ids, neighbor_idx, w1, gamma1, beta1,
           w2, gamma2, beta2):
    feats = np.asarray(feats, dtype=np.float32)
    batch_ids = np.asarray(batch_ids)
    neighbor_idx = np.asarray(neighbor_idx)
    w1 = np.asarray(w1, dtype=np.float32)
    w2 = np.asarray(w2, dtype=np.float32)
    gamma1 = np.asarray(gamma1, dtype=np.float32).reshape(-1)
    beta1 = np.asarray(beta1, dtype=np.float32).reshape(-1)
    gamma2 = np.asarray(gamma2, dtype=np.float32).reshape(-1)
    beta2 = np.asarray(beta2, dtype=np.float32).reshape(-1)

    ok = (feats.shape == (N, C) and neighbor_idx.shape == (KVOL, N)
          and np.array_equal(batch_ids,
                             np.repeat(np.arange(B, dtype=batch_ids.dtype),
                                       PER)))
    plan = _build_plan(neighbor_idx) if ok else None
    if plan is None:
        return _numpy_ref(feats, batch_ids, neighbor_idx, w1, gamma1, beta1,
                          w2, gamma2, beta2)

    runs = tuple(plan["runs"])
    npad = plan["npad"]
    nchunks = plan["nchunks"]
    nD, nC = plan["nD"], plan["nC"]
    win_sc = tuple(tuple(x) for x in plan["win_sc"])
    tc_sc = tuple(tuple(x) for x in plan["tc_sc"])
    nsp = len(plan["sparse_ks"])

    key = (runs, npad, nD, nC, win_sc, tc_sc, nsp)
    if key not in _prog_cache:
        _prog_cache[key] = _build_nc(list(runs), npad, nchunks, nD, nC,
                                     [list(x) for x in win_sc],
                                     [list(x) for x in tc_sc], nsp)
    nc = _prog_cache[key]

    w_id1 = np.zeros((C, C), dtype=np.float32)
    w_id2 = np.zeros((C, C), dtype=np.float32)
    for k in plan["identity_ks"]:
        w_id1 += w1[k]
        w_id2 += w2[k]
    sparse_ks = plan["sparse_ks"]
    # host layout [cin, wi, cout] so the DMA is partition-contiguous
    wsp1 = np.ascontiguousarray(
        w1[sparse_ks].transpose(1, 0, 2)).astype(np.float16)
    wsp2 = np.ascontiguousarray(
        w2[sparse_ks].transpose(1, 0, 2)).astype(np.float16)

    in_maps = []
    for c in range(NCORES):
        rows = np.zeros((NROWS, C), dtype=np.float16)
        rows[:PER] = feats[c * PER:(c + 1) * PER].astype(np.float16)
        m = dict(
            rows1=rows,
            wid1=w_id1.astype(np.float16),
            wid2=w_id2.astype(np.float16),
            wsp1=wsp1,
            wsp2=wsp2,
            dmat=np.ascontiguousarray(
                plan["dmat"][c].transpose(1, 0, 2)),
            cmat=np.ascontiguousarray(
                plan["cmat"][c].transpose(1, 0, 2)),
            gidx=_wrap16(plan["gsrc"][c]),
            ident=np.eye(C, dtype=np.float16),
            gam1=gamma1.reshape(C, 1),
            bet1=beta1.reshape(C, 1),
            gam2=gamma2.reshape(C, 1),
            bet2=beta2.reshape(C, 1),
        )
        in_maps.append(m)

    from concourse.bass_utils import run_bass_kernel_spmd
    res = run_bass_kernel_spmd(nc, in_maps, core_ids=list(range(NCORES)))
    global _last_results
    _last_results = res

    out = np.empty((N, C), dtype=np.float32)
    for c in range(NCORES):
        out[c * PER:(c + 1) * PER] = \
            res.results[c]["out_ft"].astype(np.float32).T
    return out
